# revision 13
# baseline (speedup 1.0000x reference)
"""Local (windowed) attention Trainium2 kernel — v4.

Reference semantics (hardcoded, matching the nn.Module):
  q,k,v: [4, 16, 4096, 64] fp32. Windows of 128 along the sequence axis.
  Each query window attends to [prev window ; own window] (256 keys).
  Window -1 is PAD: k and v VALUES filled with -1.0 (not masked!).
  out = softmax(q*dh^-0.5 @ k_cat^T) @ v_cat.

Distribution: shard the fused (b*h)=64 axis across 8 NeuronCores, 8 rows
each; attention is window-local so no cross-core communication.

v4 design (why it looks like this):
  * All on-chip matmul operands are fp16 at base_partition 0: fp32 PE
    matmuls run at 1/4 rate, and operands at base_partition 64 crash the
    device at scale, so the stacked-pair DMA-transpose layout of v3 is
    out. Scores accumulate in fp32 PSUM; end-to-end rel err ~1e-3.
  * q/k are transposed per-window on TensorE (fp16 transpose is full
    rate) into a [64, 512] PSUM staging tile (4 windows), then one DVE
    copy per 4 windows lands them in contiguous [64, 4096] qT/kT.
  * sim matmuls are window-paired: stationary kT_w streams
    [qT_w | qT_{w+1}] (N=256), producing chunks (c1 of w | c0 of w+1);
    two such pair-results share a [128,512] PSUM bank so one Exp
    activation covers 4 chunks (amortizes ACT overhead).
  * simT layout [keys, queries]: softmax denominator comes from a
    ones-column appended to V; normalization is deferred and batched
    per-bh (strided extraction + one reciprocal + one broadcast mul).
  * No max-subtraction: scores are ~N(0,1) here; exp is safe in fp32.
"""

import sys

sys.path.insert(0, "/opt/trn_rl_repo")

from contextlib import ExitStack

import numpy as np

import concourse.bass as bass
import concourse.tile as tile
from concourse import bacc, mybir
from concourse.bass_utils import run_bass_kernel_spmd
from concourse.masks import make_identity

B, H, N, DH = 4, 16, 4096, 64
WIN = 128
W = N // WIN  # 32 windows
NCORES = 8
BH = B * H
BH_PER_CORE = BH // NCORES  # 8
F32 = mybir.dt.float32
F16 = mybir.dt.float16
SCALE = DH ** -0.5  # 0.125
DMA_CHUNK = 8
EXPF = mybir.ActivationFunctionType.Exp


def _build(nc):
    q = nc.dram_tensor("q", [BH_PER_CORE, N, DH], F32, kind="ExternalInput")
    k = nc.dram_tensor("k", [BH_PER_CORE, N, DH], F32, kind="ExternalInput")
    v = nc.dram_tensor("v", [BH_PER_CORE, N, DH], F32, kind="ExternalInput")
    out = nc.dram_tensor("out", [BH_PER_CORE, N, DH], F32, kind="ExternalOutput")

    with ExitStack() as ctx:
        tc = ctx.enter_context(tile.TileContext(nc))

        singles = ctx.enter_context(tc.tile_pool(name="singles", bufs=1))
        io = ctx.enter_context(tc.tile_pool(name="io", bufs=2))
        tbh = ctx.enter_context(tc.tile_pool(name="tbh", bufs=2))
        esb = ctx.enter_context(tc.tile_pool(name="esb", bufs=4))
        nsb = ctx.enter_context(tc.tile_pool(name="nsb", bufs=2))
        ps_tr = ctx.enter_context(tc.tile_pool(name="ps_tr", bufs=3, space="PSUM"))
        ps_sim = ctx.enter_context(tc.tile_pool(name="ps_sim", bufs=3, space="PSUM"))
        ps_out = ctx.enter_context(tc.tile_pool(name="ps_out", bufs=2, space="PSUM"))

        ident = singles.tile([128, 128], F16)
        make_identity(nc, ident)
        kneg = singles.tile([64, WIN], F16)
        nc.vector.memset(kneg, -1.0)
        vneg = singles.tile([WIN, DH + 1], F16)
        nc.vector.memset(vneg, -1.0)
        nc.vector.memset(vneg[:, DH:DH + 1], 1.0)

        for j in range(BH_PER_CORE):
            q_ap = q[j].rearrange("(w p) d -> p w d", p=WIN)
            k_ap = k[j].rearrange("(w p) d -> p w d", p=WIN)
            v_ap = v[j].rearrange("(w p) d -> p w d", p=WIN)
            o_ap = out[j].rearrange("(w p) d -> p w d", p=WIN)

            qt = io.tile([WIN, W, DH], F32, tag="qtile")
            kt = io.tile([WIN, W, DH], F32, tag="ktile")
            vt = io.tile([WIN, W, DH], F32, tag="vtile")
            qh = io.tile([WIN, W, DH], F16, tag="qh")
            kh = io.tile([WIN, W, DH], F16, tag="kh")
            vh = io.tile([WIN, W, DH + 1], F16, tag="vh")
            ot = io.tile([WIN, W, DH], F32, tag="otile")
            for c in range(0, W, DMA_CHUNK):
                s = slice(c, c + DMA_CHUNK)
                nc.sync.dma_start(out=qt[:, s, :], in_=q_ap[:, s, :])
                nc.scalar.dma_start(out=kt[:, s, :], in_=k_ap[:, s, :])
                nc.gpsimd.dma_start(out=vt[:, s, :], in_=v_ap[:, s, :])
                nc.gpsimd.tensor_copy(qh[:, s, :], qt[:, s, :])
                nc.gpsimd.tensor_copy(kh[:, s, :], kt[:, s, :])
                nc.vector.memset(vh[:, s, DH:DH + 1], 1.0)
                nc.gpsimd.tensor_copy(vh[:, s, 0:DH], vt[:, s, :])

            # Contiguous transposed views [dh=64, W*128] via per-window
            # TensorE transposes staged 4-at-a-time in [64, 512] PSUM.
            qT = tbh.tile([DH, W * WIN], F16, tag="qT")
            kT = tbh.tile([DH, W * WIN], F16, tag="kT")
            for c in range(0, W, 4):
                trq = ps_tr.tile([DH, 4 * WIN], F16, tag="trp")
                trk = ps_tr.tile([DH, 4 * WIN], F16, tag="trp")
                for m in range(4):
                    nc.tensor.transpose(trq[:, m * WIN:(m + 1) * WIN],
                                        qh[:, c + m, :], ident)
                    nc.tensor.transpose(trk[:, m * WIN:(m + 1) * WIN],
                                        kh[:, c + m, :], ident)
                nc.vector.tensor_copy(qT[:, c * WIN:(c + 4) * WIN], trq)
                nc.vector.tensor_copy(kT[:, c * WIN:(c + 4) * WIN], trk)

            denb = nsb.tile([WIN, W], F32, tag="denb")
            rden = nsb.tile([WIN, W], F32, tag="rden")

            for g in range(W // 2):  # 2 windows per iteration
                w0, w1 = 2 * g, 2 * g + 1
                # simT chunk layout in one PSUM bank:
                # cols 0:128   = kT_{w0-1}.T qT_{w0}        (c0 of w0)
                # cols 128:384 = kT_{w0}.T [qT_{w0}|qT_{w1}] (c1 w0, c0 w1)
                # cols 384:512 = kT_{w1}.T qT_{w1}          (c1 of w1)
                simg = ps_sim.tile([WIN, 4 * WIN], F32, tag="sim")
                lhs0 = kneg[:] if g == 0 else kT[:, (w0 - 1) * WIN:w0 * WIN]
                nc.tensor.matmul(simg[:, 0:128], lhsT=lhs0,
                                 rhs=qT[:, w0 * WIN:(w0 + 1) * WIN],
                                 start=True, stop=True)
                nc.tensor.matmul(simg[:, 128:384],
                                 lhsT=kT[:, w0 * WIN:(w0 + 1) * WIN],
                                 rhs=qT[:, w0 * WIN:(w0 + 2) * WIN],
                                 start=True, stop=True)
                nc.tensor.matmul(simg[:, 384:512],
                                 lhsT=kT[:, w1 * WIN:(w1 + 1) * WIN],
                                 rhs=qT[:, w1 * WIN:(w1 + 1) * WIN],
                                 start=True, stop=True)

                expg = esb.tile([WIN, 4 * WIN], F16, tag="exp")
                nc.scalar.activation(expg, simg, EXPF, scale=SCALE)

                # AV: w0 chunks are exp cols (0:128, 128:256);
                # w1 chunks are (256:384, 384:512)
                oun = ps_out.tile([WIN, 2 * (DH + 1)], F32, tag="oun")
                nc.tensor.matmul(oun[:, 0:DH + 1], lhsT=expg[:, 0:128],
                                 rhs=vneg[:] if g == 0 else vh[:, w0 - 1, :],
                                 start=True, stop=False)
                nc.tensor.matmul(oun[:, 0:DH + 1], lhsT=expg[:, 128:256],
                                 rhs=vh[:, w0, :], start=False, stop=True)
                nc.tensor.matmul(oun[:, DH + 1:], lhsT=expg[:, 256:384],
                                 rhs=vh[:, w0, :], start=True, stop=False)
                nc.tensor.matmul(oun[:, DH + 1:], lhsT=expg[:, 384:512],
                                 rhs=vh[:, w1, :], start=False, stop=True)

                nc.vector.tensor_copy(
                    ot[:, w0:w1 + 1, :],
                    oun.rearrange("p (m c) -> p m c", m=2)[:, :, 0:DH])
                nc.vector.tensor_copy(
                    denb[:, w0:w1 + 1],
                    oun.rearrange("p (m c) -> p m c", m=2)[:, :, DH])

            nc.vector.reciprocal(rden, denb)
            rd = rden[:]
            bcast = bass.AP(tensor=rd.tensor, offset=rd.offset,
                            ap=[rd.ap[0], rd.ap[1], [0, DH]])
            nc.vector.tensor_mul(ot[:], ot[:], bcast)

            for c in range(0, W, DMA_CHUNK):
                s = slice(c, c + DMA_CHUNK)
                nc.gpsimd.dma_start(out=o_ap[:, s, :], in_=ot[:, s, :])

    nc.finalize()
    return nc


_NC_CACHE = None


def _get_nc():
    global _NC_CACHE
    if _NC_CACHE is None:
        nc = bacc.Bacc("TRN2", target_bir_lowering=False, debug=False,
                       num_devices=NCORES)
        _NC_CACHE = _build(nc)
    return _NC_CACHE


def kernel(q, k, v, **_unused):
    q = np.ascontiguousarray(np.asarray(q, dtype=np.float32)).reshape(BH, N, DH)
    k = np.ascontiguousarray(np.asarray(k, dtype=np.float32)).reshape(BH, N, DH)
    v = np.ascontiguousarray(np.asarray(v, dtype=np.float32)).reshape(BH, N, DH)

    nc = _get_nc()
    in_maps = []
    for c in range(NCORES):
        s = slice(c * BH_PER_CORE, (c + 1) * BH_PER_CORE)
        in_maps.append({"q": q[s], "k": k[s], "v": v[s]})

    res = run_bass_kernel_spmd(nc, in_maps, list(range(NCORES))).results
    full = np.concatenate([res[c]["out"] for c in range(NCORES)], axis=0)
    return full.reshape(B, H, N, DH)


# revision 14
# speedup vs baseline: 1.1968x; 1.1968x over previous
"""Local (windowed) attention Trainium2 kernel — v4.

Reference semantics (hardcoded, matching the nn.Module):
  q,k,v: [4, 16, 4096, 64] fp32. Windows of 128 along the sequence axis.
  Each query window attends to [prev window ; own window] (256 keys).
  Window -1 is PAD: k and v VALUES filled with -1.0 (not masked!).
  out = softmax(q*dh^-0.5 @ k_cat^T) @ v_cat.

Distribution: shard the fused (b*h)=64 axis across 8 NeuronCores, 8 rows
each; attention is window-local so no cross-core communication.

v4 design (why it looks like this):
  * All on-chip matmul operands are fp16 at base_partition 0: fp32 PE
    matmuls run at 1/4 rate, and operands at base_partition 64 crash the
    device at scale, so the stacked-pair DMA-transpose layout of v3 is
    out. Scores accumulate in fp32 PSUM; end-to-end rel err ~1e-3.
  * q/k are transposed per-window on TensorE (fp16 transpose is full
    rate) into a [64, 512] PSUM staging tile (4 windows), then one DVE
    copy per 4 windows lands them in contiguous [64, 4096] qT/kT.
  * sim matmuls are window-paired: stationary kT_w streams
    [qT_w | qT_{w+1}] (N=256), producing chunks (c1 of w | c0 of w+1);
    two such pair-results share a [128,512] PSUM bank so one Exp
    activation covers 4 chunks (amortizes ACT overhead).
  * simT layout [keys, queries]: softmax denominator comes from a
    ones-column appended to V; normalization is deferred and batched
    per-bh (strided extraction + one reciprocal + one broadcast mul).
  * No max-subtraction: scores are ~N(0,1) here; exp is safe in fp32.
"""

import sys

sys.path.insert(0, "/opt/trn_rl_repo")

from contextlib import ExitStack

import numpy as np

import concourse.bass as bass
import concourse.tile as tile
from concourse import bacc, mybir
from concourse.bass_utils import run_bass_kernel_spmd
from concourse.masks import make_identity

B, H, N, DH = 4, 16, 4096, 64
WIN = 128
W = N // WIN  # 32 windows
NCORES = 8
BH = B * H
BH_PER_CORE = BH // NCORES  # 8
F32 = mybir.dt.float32
F16 = mybir.dt.float16
SCALE = DH ** -0.5  # 0.125
DMA_CHUNK = 8
EXPF = mybir.ActivationFunctionType.Exp


def _build(nc):
    q = nc.dram_tensor("q", [BH_PER_CORE, N, DH], F32, kind="ExternalInput")
    k = nc.dram_tensor("k", [BH_PER_CORE, N, DH], F32, kind="ExternalInput")
    v = nc.dram_tensor("v", [BH_PER_CORE, N, DH], F32, kind="ExternalInput")
    out = nc.dram_tensor("out", [BH_PER_CORE, N, DH], F32, kind="ExternalOutput")

    with ExitStack() as ctx:
        tc = ctx.enter_context(tile.TileContext(nc))

        singles = ctx.enter_context(tc.tile_pool(name="singles", bufs=1))
        io = ctx.enter_context(tc.tile_pool(name="io", bufs=2))
        tbh = ctx.enter_context(tc.tile_pool(name="tbh", bufs=2))
        esb = ctx.enter_context(tc.tile_pool(name="esb", bufs=4))
        nsb = ctx.enter_context(tc.tile_pool(name="nsb", bufs=2))
        ps_tr = ctx.enter_context(tc.tile_pool(name="ps_tr", bufs=3, space="PSUM"))
        ps_sim = ctx.enter_context(tc.tile_pool(name="ps_sim", bufs=3, space="PSUM"))
        ps_out = ctx.enter_context(tc.tile_pool(name="ps_out", bufs=2, space="PSUM"))

        ident = singles.tile([128, 128], F16)
        make_identity(nc, ident)
        kneg = singles.tile([64, WIN], F16)
        nc.vector.memset(kneg, -1.0)
        vneg = singles.tile([WIN, DH + 1], F16)
        nc.vector.memset(vneg, -1.0)
        nc.vector.memset(vneg[:, DH:DH + 1], 1.0)

        for j in range(BH_PER_CORE):
            q_ap = q[j].rearrange("(w p) d -> p w d", p=WIN)
            k_ap = k[j].rearrange("(w p) d -> p w d", p=WIN)
            v_ap = v[j].rearrange("(w p) d -> p w d", p=WIN)
            o_ap = out[j].rearrange("(w p) d -> p w d", p=WIN)

            qt = io.tile([WIN, W, DH], F32, tag="qtile")
            kt = io.tile([WIN, W, DH], F32, tag="ktile")
            vt = io.tile([WIN, W, DH], F32, tag="vtile")
            qh = io.tile([WIN, W, DH], F16, tag="qh")
            kh = io.tile([WIN, W, DH], F16, tag="kh")
            vh = io.tile([WIN, W, DH + 1], F16, tag="vh")
            ot = io.tile([WIN, W, DH], F32, tag="otile")
            for c in range(0, W, DMA_CHUNK):
                s = slice(c, c + DMA_CHUNK)
                nc.sync.dma_start(out=qt[:, s, :], in_=q_ap[:, s, :])
                nc.scalar.dma_start(out=kt[:, s, :], in_=k_ap[:, s, :])
                nc.gpsimd.dma_start(out=vt[:, s, :], in_=v_ap[:, s, :])
                nc.vector.tensor_copy(qh[:, s, :], qt[:, s, :])
                nc.vector.tensor_copy(kh[:, s, :], kt[:, s, :])
                nc.vector.memset(vh[:, s, DH:DH + 1], 1.0)
                nc.vector.tensor_copy(vh[:, s, 0:DH], vt[:, s, :])

            # Contiguous transposed views [dh=64, W*128] via per-window
            # TensorE transposes staged 4-at-a-time in [64, 512] PSUM.
            # Emitted interleaved with the compute groups: transpose-mode
            # does not warm the HAM clock gate, but dense neighboring
            # matmul traffic keeps it open so transposes run at 2.4 GHz.
            qT = tbh.tile([DH, W * WIN], F16, tag="qT")
            kT = tbh.tile([DH, W * WIN], F16, tag="kT")

            def transpose4(c):
                trq = ps_tr.tile([DH, 4 * WIN], F16, tag="trp")
                trk = ps_tr.tile([DH, 4 * WIN], F16, tag="trp")
                for m in range(4):
                    nc.tensor.transpose(trq[:, m * WIN:(m + 1) * WIN],
                                        qh[:, c + m, :], ident)
                    nc.tensor.transpose(trk[:, m * WIN:(m + 1) * WIN],
                                        kh[:, c + m, :], ident)
                nc.vector.tensor_copy(qT[:, c * WIN:(c + 4) * WIN], trq)
                nc.vector.tensor_copy(kT[:, c * WIN:(c + 4) * WIN], trk)

            transpose4(0)
            transpose4(4)

            for g in range(W // 2):  # 2 windows per iteration
                c_need = 8 + 2 * g  # stay 2 groups ahead of the sims
                if c_need % 4 == 0 and c_need < W:
                    transpose4(c_need)
                w0, w1 = 2 * g, 2 * g + 1
                # simT chunk layout in one PSUM bank:
                # cols 0:128   = kT_{w0-1}.T qT_{w0}        (c0 of w0)
                # cols 128:384 = kT_{w0}.T [qT_{w0}|qT_{w1}] (c1 w0, c0 w1)
                # cols 384:512 = kT_{w1}.T qT_{w1}          (c1 of w1)
                simg = ps_sim.tile([WIN, 4 * WIN], F32, tag="sim")
                lhs0 = kneg[:] if g == 0 else kT[:, (w0 - 1) * WIN:w0 * WIN]
                nc.tensor.matmul(simg[:, 0:128], lhsT=lhs0,
                                 rhs=qT[:, w0 * WIN:(w0 + 1) * WIN],
                                 start=True, stop=True)
                nc.tensor.matmul(simg[:, 128:384],
                                 lhsT=kT[:, w0 * WIN:(w0 + 1) * WIN],
                                 rhs=qT[:, w0 * WIN:(w0 + 2) * WIN],
                                 start=True, stop=True)
                nc.tensor.matmul(simg[:, 384:512],
                                 lhsT=kT[:, w1 * WIN:(w1 + 1) * WIN],
                                 rhs=qT[:, w1 * WIN:(w1 + 1) * WIN],
                                 start=True, stop=True)

                expg = esb.tile([WIN, 4 * WIN], F16, tag="exp")
                nc.scalar.activation(expg, simg, EXPF, scale=SCALE)

                # AV: w0 chunks are exp cols (0:128, 128:256);
                # w1 chunks are (256:384, 384:512)
                oun = ps_out.tile([WIN, 2 * (DH + 1)], F32, tag="oun")
                nc.tensor.matmul(oun[:, 0:DH + 1], lhsT=expg[:, 0:128],
                                 rhs=vneg[:] if g == 0 else vh[:, w0 - 1, :],
                                 start=True, stop=False)
                nc.tensor.matmul(oun[:, 0:DH + 1], lhsT=expg[:, 128:256],
                                 rhs=vh[:, w0, :], start=False, stop=True)
                nc.tensor.matmul(oun[:, DH + 1:], lhsT=expg[:, 256:384],
                                 rhs=vh[:, w0, :], start=True, stop=False)
                nc.tensor.matmul(oun[:, DH + 1:], lhsT=expg[:, 384:512],
                                 rhs=vh[:, w1, :], start=False, stop=True)

                # fused normalize: reciprocal of the ones-column, then
                # one broadcast multiply extracts+normalizes both windows
                rden = nsb.tile([WIN, 2], F32, tag="rden")
                nc.vector.reciprocal(
                    rden, oun.rearrange("p (m c) -> p m c", m=2)[:, :, DH])
                rd = rden[:]
                bcast = bass.AP(tensor=rd.tensor, offset=rd.offset,
                                ap=[rd.ap[0], rd.ap[1], [0, DH]])
                nc.vector.tensor_mul(
                    ot[:, w0:w1 + 1, :],
                    oun.rearrange("p (m c) -> p m c", m=2)[:, :, 0:DH],
                    bcast)

            for c in range(0, W, DMA_CHUNK):
                s = slice(c, c + DMA_CHUNK)
                nc.gpsimd.dma_start(out=o_ap[:, s, :], in_=ot[:, s, :])

    nc.finalize()
    return nc


_NC_CACHE = None


def _get_nc():
    global _NC_CACHE
    if _NC_CACHE is None:
        nc = bacc.Bacc("TRN2", target_bir_lowering=False, debug=False,
                       num_devices=NCORES)
        _NC_CACHE = _build(nc)
    return _NC_CACHE


def kernel(q, k, v, **_unused):
    q = np.ascontiguousarray(np.asarray(q, dtype=np.float32)).reshape(BH, N, DH)
    k = np.ascontiguousarray(np.asarray(k, dtype=np.float32)).reshape(BH, N, DH)
    v = np.ascontiguousarray(np.asarray(v, dtype=np.float32)).reshape(BH, N, DH)

    nc = _get_nc()
    in_maps = []
    for c in range(NCORES):
        s = slice(c * BH_PER_CORE, (c + 1) * BH_PER_CORE)
        in_maps.append({"q": q[s], "k": k[s], "v": v[s]})

    res = run_bass_kernel_spmd(nc, in_maps, list(range(NCORES))).results
    full = np.concatenate([res[c]["out"] for c in range(NCORES)], axis=0)
    return full.reshape(B, H, N, DH)
